# revision 36
# baseline (speedup 1.0000x reference)
"""AdaptiveLocal2DLayer forward on 8 TRN2 NeuronCores.

out[b, n] = sum_{c,h,w} x[b,c,h,w] * mask[h,w,n] * weights[c,h,w,n]
mask[h,w,n] = gy[h,n] * gx[w,n] * s[n]          (separable Gaussian)
s[n] = sqrt(H*W) / sqrt(sum_h gy^2 * sum_w gx^2)

Sharding: neuron axis N=1024 split over 8 cores (128 each). Weights are the
dominant traffic (201MB) and are read exactly once chip-wide; x (6.3MB) is
replicated. No collectives.

Per-core algorithm (everything on device except layout prep):
  - build gx^T[n',w], gy^T[n',h], s[n'] from mu/sigma via DVE+ACT (exp)
  - gys^T[n',h] = gy^T * s ; gx[w,n'] via PE transpose
  - for each (c, h-block): DMA weights chunk [w=128, 16h, 128n'],
    one DVE multiply by gx (broadcast over h), then per h:
    matmul(lhsT=gx*W [w,n'], rhs=x^T [w,b]) accumulated over c in PSUM,
    PSUM->SBUF copy on ACT fused with the per-partition gys[h] scale.
  - final DVE reduce over h, DMA out [n',b].
"""

import numpy as np

import concourse.bass as bass
import concourse.mybir as mybir
import concourse.tile as tile
from concourse import bacc
from concourse.bass_utils import run_bass_kernel_spmd
from concourse.masks import make_identity

B, C, H, W = 32, 3, 128, 128
N = 1024
NCORES = 8
NS = N // NCORES  # 128 neurons per core
HB = 16           # h-block size
NHB = H // HB

F32 = mybir.dt.float32
BF16 = mybir.dt.bfloat16
AF = mybir.ActivationFunctionType

# h-blocks: small first (fast pipeline fill) and small last (short kernel
# tail after the final DMA byte); big blocks mid-stream for DMA efficiency.
BLOCKS = [4] + [16] * 7 + [4, 4, 4]
assert sum(BLOCKS) == H

LAST_RESULT = None  # BassKernelResults stash for test harness
LAST_NC = None
LAST_IN_MAPS = None


def build_nc() -> bass.Bass:
    nc = bacc.Bacc("TRN2", target_bir_lowering=False, num_swdge_queues=4)

    # chunk-linear flat layouts: one (c, h-block) chunk = one contiguous span
    wt_d = nc.dram_tensor("wt", [C * H * W * NS], F32, kind="ExternalInput")
    xt_d = nc.dram_tensor("xt", [C * H * W * B], BF16, kind="ExternalInput")
    mux_d = nc.dram_tensor("mu_x", [NS], F32, kind="ExternalInput")
    muy_d = nc.dram_tensor("mu_y", [NS], F32, kind="ExternalInput")
    sgx_d = nc.dram_tensor("sigma_x", [NS], F32, kind="ExternalInput")
    sgy_d = nc.dram_tensor("sigma_y", [NS], F32, kind="ExternalInput")
    grid_d = nc.dram_tensor("grid", [W], F32, kind="ExternalInput")
    ident_d = nc.dram_tensor("ident", [128, 128], F32, kind="ExternalInput")
    out_d = nc.dram_tensor("out", [NS, B], F32, kind="ExternalOutput")

    with tile.TileContext(nc) as tc:
        with (
            tc.tile_pool(name="singles", bufs=1) as singles,
            tc.tile_pool(name="wpool", bufs=10) as wpool,
            tc.tile_pool(name="xbpool", bufs=10) as xbpool,
            tc.tile_pool(name="ppool", bufs=7, space="PSUM") as ppool,
            tc.tile_pool(name="tpsum", bufs=1, space="PSUM") as tpsum,
        ):
            # ---------------- mask construction ----------------
            grid_b = singles.tile([128, W], F32)
            nc.sync.dma_start(
                out=grid_b,
                in_=bass.AP(tensor=grid_d, offset=0, ap=[[0, 128], [1, W]]),
            )

            def col_load(dram):
                t = singles.tile([128, 1], F32, tag=f"col_{dram.name}")
                nc.sync.dma_start(
                    out=t, in_=bass.AP(tensor=dram, offset=0, ap=[[1, 128], [1, 1]])
                )
                return t

            mux_c = col_load(mux_d)
            muy_c = col_load(muy_d)
            sgx_c = col_load(sgx_d)
            sgy_c = col_load(sgy_d)

            inv_sgx = singles.tile([128, 1], F32)
            nc.vector.reciprocal(out=inv_sgx, in_=sgx_c)
            inv_sgy = singles.tile([128, 1], F32)
            nc.vector.reciprocal(out=inv_sgy, in_=sgy_c)

            # gT[n', u] = exp(-.5*((grid[u]-mu[n'])/sigma[n'])^2), row-sum of squares
            def gauss(mu_c, inv_sg, tag):
                z = singles.tile([128, W], F32, tag=f"z_{tag}")
                nc.vector.tensor_scalar(
                    out=z, in0=grid_b, scalar1=mu_c, scalar2=inv_sg,
                    op0=mybir.AluOpType.subtract, op1=mybir.AluOpType.mult,
                )
                nc.vector.tensor_mul(out=z, in0=z, in1=z)
                g = singles.tile([128, W], F32, tag=f"g_{tag}")
                nc.scalar.activation(out=g, in_=z, func=AF.Exp, scale=-0.5)
                ssq = singles.tile([128, 1], F32, tag=f"ssq_{tag}")
                trash = singles.tile([128, W], F32, tag="trash")
                nc.scalar.activation(
                    out=trash, in_=g, func=AF.Square, accum_out=ssq
                )
                return g, ssq

            gxT, sx = gauss(mux_c, inv_sgx, "x")
            gyT, sy = gauss(muy_c, inv_sgy, "y")

            # s[n'] = sqrt(H*W)/sqrt(sx*sy) = 1/sqrt(sx*sy/(H*W))
            s_col = singles.tile([128, 1], F32)
            nc.vector.tensor_mul(out=s_col, in0=sx, in1=sy)
            nc.scalar.activation(
                out=s_col, in_=s_col, func=AF.Sqrt, scale=1.0 / (H * W)
            )
            nc.vector.reciprocal(out=s_col, in_=s_col)

            gysT = singles.tile([128, H], F32)  # [n', h] = gy^T * s
            nc.vector.tensor_scalar_mul(out=gysT, in0=gyT, scalar1=s_col)

            # gx[w, n'] = transpose(gxT) via PE
            ident = singles.tile([128, 128], F32)
            nc.sync.dma_start(out=ident, in_=ident_d[:, :])
            gx_ps = tpsum.tile([128, 128], F32)
            nc.tensor.transpose(gx_ps, gxT, ident)
            gx_sb = singles.tile([128, NS], BF16)
            nc.vector.tensor_copy(out=gx_sb, in_=gx_ps)
            gx_bc = gx_sb.rearrange("w (o n) -> w o n", o=1).to_broadcast(
                [128, HB, NS]
            )

            # t1[n', h, b]: per-h matmul results; scaled in place per block
            t1_sb = singles.tile([128, H, B], F32)
            out_acc = singles.tile([128, B], F32)

            # ---------------- main streaming loop ----------------
            h0 = 0
            for bi, hbs in enumerate(BLOCKS):
                wgs = []
                xbs = []
                for c in range(C):
                    # SWDGE DMA casts f32 -> bf16 in flight (HBM reads f32)
                    wg_t = wpool.tile([128, hbs * NS], BF16, tag="wt")
                    woff = (c * H + h0) * W * NS
                    nc.gpsimd.dma_start(
                        out=wg_t,
                        in_=bass.AP(
                            tensor=wt_d, offset=woff,
                            ap=[[hbs * NS, 128], [NS, hbs], [1, NS]],
                        ),
                    )
                    wg_t = wg_t.rearrange("w (h n) -> w h n", h=hbs)
                    # mask multiply in place, bf16 x bf16 (2x DVE mode)
                    gxb = gx_sb.rearrange("w (o n) -> w o n", o=1).to_broadcast(
                        [128, hbs, NS]
                    )
                    nc.vector.tensor_mul(out=wg_t, in0=wg_t, in1=gxb)
                    wgs.append(wg_t)
                    xb_t = xbpool.tile([128, hbs * B], BF16, tag="xb")
                    xoff = (c * H + h0) * W * B
                    nc.sync.dma_start(
                        out=xb_t,
                        in_=bass.AP(
                            tensor=xt_d, offset=xoff,
                            ap=[[hbs * B, 128], [B, hbs], [1, B]],
                        ),
                    )
                    xbs.append(xb_t.rearrange("w (h b) -> w h b", h=hbs))
                hg0 = 0
                while hg0 < hbs:
                    hg = min(8, hbs - hg0)
                    pt = ppool.tile([128, 8, B], F32)
                    for hl in range(hg):
                        for c in range(C):
                            nc.tensor.matmul(
                                pt[:, hl, :],
                                lhsT=wgs[c][:, hg0 + hl, :],
                                rhs=xbs[c][:, hg0 + hl, :],
                                start=(c == 0),
                                stop=(c == C - 1),
                            )
                    nc.scalar.activation(
                        out=t1_sb[:, h0 + hg0 : h0 + hg0 + hg, :],
                        in_=pt[:, :hg, :], func=AF.Copy,
                    )
                    hg0 += hg
                # scale this block by gy*s in place (overlaps the stream),
                # reduce over the block's h rows, accumulate
                gys_bc = gysT[:, h0 : h0 + hbs].rearrange(
                    "n (h o) -> n h o", o=1
                ).to_broadcast([128, hbs, B])
                t1h = t1_sb[:, h0 : h0 + hbs, :]
                nc.vector.tensor_mul(out=t1h, in0=t1h, in1=gys_bc)
                red_out = out_acc if bi == 0 else singles.tile(
                    [128, B], F32, tag=f"red_{bi}"
                )
                nc.vector.tensor_reduce(
                    out=red_out,
                    in_=t1h.rearrange("n h b -> n b h"),
                    axis=mybir.AxisListType.X,
                    op=mybir.AluOpType.add,
                )
                if bi > 0:
                    nc.vector.tensor_add(
                        out=out_acc, in0=out_acc, in1=red_out
                    )
                h0 += hbs

            nc.sync.dma_start(out=out_d[:, :], in_=out_acc)

    nc.compile()
    return nc


def prep_in_maps(x, mu_x, mu_y, sigma_x, sigma_y, weights):
    import ml_dtypes

    starts = []
    h0 = 0
    for hbs in BLOCKS:
        starts.append((h0, hbs))
        h0 += hbs

    # flat chunk-linear x: chunk (c, h0) is [W, hbs, B], c-major then h0
    xt = np.concatenate(
        [
            np.transpose(x[:, c, h0 : h0 + hbs, :], (2, 1, 0)).ravel()
            for c in range(C)
            for h0, hbs in starts
        ]
    ).astype(ml_dtypes.bfloat16)
    grid = np.linspace(0.0, 1.0, W, dtype=np.float32)
    in_maps = []
    for k in range(NCORES):
        sl = slice(k * NS, (k + 1) * NS)
        wsh = weights[:, :, :, sl]  # [C,H,W,NS]
        # flat chunk-linear weights: chunk (c, h0) is [W, hbs, NS]
        wt = np.concatenate(
            [
                np.transpose(wsh[c, h0 : h0 + hbs], (1, 0, 2)).ravel()
                for c in range(C)
                for h0, hbs in starts
            ]
        )
        in_maps.append(
            {
                "wt": wt,
                "xt": xt,
                "mu_x": np.ascontiguousarray(mu_x[sl]),
                "mu_y": np.ascontiguousarray(mu_y[sl]),
                "sigma_x": np.ascontiguousarray(sigma_x[sl]),
                "sigma_y": np.ascontiguousarray(sigma_y[sl]),
                "grid": grid,
                "ident": np.eye(128, dtype=np.float32),
            }
        )
    return in_maps


def kernel(x, mu_x, mu_y, sigma_x, sigma_y, weights):
    global LAST_RESULT
    x = np.asarray(x, dtype=np.float32)
    mu_x = np.asarray(mu_x, dtype=np.float32)
    mu_y = np.asarray(mu_y, dtype=np.float32)
    sigma_x = np.asarray(sigma_x, dtype=np.float32)
    sigma_y = np.asarray(sigma_y, dtype=np.float32)
    weights = np.asarray(weights, dtype=np.float32)

    global LAST_NC, LAST_IN_MAPS
    nc = build_nc()
    in_maps = prep_in_maps(x, mu_x, mu_y, sigma_x, sigma_y, weights)
    res = run_bass_kernel_spmd(nc, in_maps, core_ids=list(range(NCORES)))
    LAST_RESULT = res
    LAST_NC = nc
    LAST_IN_MAPS = in_maps
    full = np.concatenate([r["out"] for r in res.results], axis=0)  # [N, B]
    return np.ascontiguousarray(full.T).reshape(B, 1, 32, 32).astype(np.float32)
